# revision 1
# baseline (speedup 1.0000x reference)
"""Trainium2 Bass kernel for nn_CRNet (gnn_message_passing).

Math (reference):
  vc   = relu(vf @ W_v1 + b_v1) @ W_v2 + b_v2                 # [B,D]
  clu  = relu(cc @ W_v1 + b_v1) @ W_v2 + b_v2                 # [K,D]
  sp   = relu(cp @ W_s1 + b_s1) @ W_s2 + b_s2                 # [C,D]
  out1[p,:] = sum_{k,e} relu((sp[p]-clu[k]) @ W_exp[e] + b_exp[e])   # [C,D]
  out2[b,c] = relu(vc[b]@Wa + out1[c]@Wb + b_r1) @ w2 + b_r2         # [B,C]

Factorization used on-device:
  A''[e] = sp @ W_exp[e] + b_exp[e]     (small matmuls, replaces the
  Dm[e]  = -(clu @ W_exp[e])             20-GFLOP [C,K,E,D] einsum)
  out1[p,d'] = sum_{k,e} relu(A''[e][p,d'] + Dm[e][k,d'])     <- fused bias+relu
  out2[b,c]  = sum_d' w2[d'] relu(VA[b,d'] + S''[c,d']) + br2 <- fused bias+relu
with fused ops laid out [d' (partitions), class/batch (free)] so the bias
is a per-partition scalar: one DVE tensor_scalar (fp16, 4x mode) or one
ACT activation per unit.  Reductions run on the PE: identity-matmul PSUM
accumulation for block1 (two k's packed per matmul, N=512), and M=32
column-tiled matmuls with replicated-w2 stationary weights for block2.

Two SPMD launches over 8 cores (collectives on this 8-core mesh measure
~35us wall each, far more than a second NEFF's ~12us fixed preamble):
  launch A: visual/semantic/cluster mappers + block1.
     block1 sharded (expert-half x cluster-quarter): 3 experts x 25
     clusters per core; VA'_T sharded over b (128 rows/core).
     Outputs per core: out1 partial [d',p] f32 and VA'_T chunk fp16.
  host: cross-core reshuffle (slice/concat only, no arithmetic):
     core i gets all 8 out1 partial slices [8, 256, 32] for its 32
     classes, plus the assembled VA'_T [256, 1024].
  launch B: sum the 8 partial slices, S''_T = Wb.T @ out1_T, block2
     c-sharded (32 classes x all 1024 b per core), + b_r2, out.
"""

import numpy as np

B, C, K, VD, SD, D, E = 1024, 256, 100, 64, 200, 256, 6
NCORES = 8
BSH = B // NCORES      # 128 b per core (visual shard)
CSH = C // NCORES      # 32 classes per core (block2 shard)
EH = 3                 # experts per core (expert half)
KQ = 25                # clusters per core (cluster quarter)
DT = 2                 # 128-partition tiles covering D=256

BLK1_ACT_EVERY = 4     # every Nth block1 fused unit goes to ACT
BLK2_ACT_EVERY = 4     # every Nth block2 fused unit goes to ACT


def _mklayout(fields):
    d, off = {}, 0
    for n, w in fields:
        d[n] = (off, w)
        off += w
    return d, off


_F32A, F32A_W = _mklayout([
    ("wv1", D), ("wv2", DT * D), ("ws2", DT * D), ("wa", DT * D),
    ("ws1a", D), ("ws1b", D), ("cpT0", C), ("cpT1", C),
    ("bv1", DT), ("bv2", DT), ("bs1", DT), ("bs2", DT), ("br1", DT)])
_F32P, F32P_W = _mklayout([
    ("vfT", BSH), ("cluT", KQ), ("wexp0", DT * D), ("wexp1", DT * D),
    ("wexp2", DT * D), ("bexp", EH * DT)])
_F32B, F32B_W = _mklayout([("wb", DT * D), ("br2", 1)])
_F16B, F16B_W = _mklayout([("w2h", DT * 32)])


def _chunked_load(nc, blob_sb, blob_dram, width):
    nchunk = max(1, round(width / 1024))
    edges = [width * i // nchunk for i in range(nchunk + 1)]
    for a, b in zip(edges[:-1], edges[1:]):
        nc.sync.dma_start(out=blob_sb[:, a:b], in_=blob_dram[:, a:b])


def _build_a():
    import concourse.bacc as bacc
    import concourse.mybir as mybir
    from concourse import tile

    f32, f16 = mybir.dt.float32, mybir.dt.float16
    AF, OP = mybir.ActivationFunctionType, mybir.AluOpType

    nc = bacc.Bacc("TRN2", target_bir_lowering=False, debug=False,
                   enable_asserts=False, num_devices=NCORES)
    blob32a = nc.dram_tensor("blob32a", [128, F32A_W], f32,
                             kind="ExternalInput").ap()
    blob32p = nc.dram_tensor("blob32p", [128, F32P_W], f32,
                             kind="ExternalInput").ap()
    idh16 = nc.dram_tensor("idh16", [128, 128], f16, kind="ExternalInput").ap()
    part = nc.dram_tensor("part", [D, C], f32, kind="ExternalOutput").ap()
    vach = nc.dram_tensor("vach", [D, BSH], f16, kind="ExternalOutput").ap()

    with tile.TileContext(nc) as tc:
        with (
            tc.tile_pool(name="const", bufs=1) as cpool,
            tc.tile_pool(name="work", bufs=3) as wpool,
            tc.tile_pool(name="h1", bufs=8) as h1pool,
            tc.tile_pool(name="ps", bufs=4, space="PSUM") as pspool,
        ):
            b32a = cpool.tile([128, F32A_W], f32, tag="b32a")
            b32p = cpool.tile([128, F32P_W], f32, tag="b32p")
            idh_sb = cpool.tile([128, 128], f16, tag="idh")
            _chunked_load(nc, b32a, blob32a, F32A_W)
            _chunked_load(nc, b32p, blob32p, F32P_W)
            nc.sync.dma_start(out=idh_sb[:], in_=idh16)

            A = lambda n: b32a[:, _F32A[n][0]:_F32A[n][0] + _F32A[n][1]]
            P = lambda n: b32p[:, _F32P[n][0]:_F32P[n][0] + _F32P[n][1]]
            wv1_sb, wv2_sb, ws2_sb, wa_sb = A("wv1"), A("wv2"), A("ws2"), A("wa")
            ws1a_sb, ws1b_sb = A("ws1a"), A("ws1b")
            cpT0_sb, cpT1_sb = A("cpT0"), A("cpT1")
            bv1_sb, bv2_sb, bs1_sb = A("bv1"), A("bv2"), A("bs1")
            bs2_sb, br1_sb = A("bs2"), A("br1")
            vfT_sb, cluT_sb = P("vfT"), P("cluT")
            wexp_sb = [P(f"wexp{e}") for e in range(EH)]
            bexp_sb = P("bexp")

            def wslice(wsb, kt, mt):
                return wsb[:, kt * D + mt * 128: kt * D + mt * 128 + 128]

            def mapper_visual(inT_ap, n, tag):
                r1 = wpool.tile([128, DT * n], f32, tag=f"{tag}_r1",
                                name=f"{tag}_r1")
                for mt in range(DT):
                    ps = pspool.tile([128, 512], f32, tag="ps_map",
                                     name=f"{tag}_ps{mt}")
                    nc.tensor.matmul(ps[:, :n], wv1_sb[:VD, mt * 128:(mt + 1) * 128],
                                     inT_ap, start=True, stop=True)
                    nc.scalar.activation(r1[:, mt * n:(mt + 1) * n], ps[:, :n],
                                         AF.Relu, bias=bv1_sb[:, mt:mt + 1])
                outs = []
                for mt in range(DT):
                    ps = pspool.tile([128, 512], f32, tag="ps_map",
                                     name=f"{tag}_ps2{mt}")
                    for kt in range(DT):
                        nc.tensor.matmul(ps[:, :n], wslice(wv2_sb, kt, mt),
                                         r1[:, kt * n:(kt + 1) * n],
                                         start=(kt == 0), stop=(kt == DT - 1))
                    o = wpool.tile([128, n], f32, tag=f"{tag}_o{mt}",
                                   name=f"{tag}_o{mt}")
                    nc.scalar.activation(o[:], ps[:, :n], AF.Identity,
                                         bias=bv2_sb[:, mt:mt + 1])
                    outs.append(o)
                return outs

            # visual mapper over [vfT | cluT] (adjacent in blob32p):
            # cols 0:BSH = this core's b-shard, BSH:BSH+KQ = cluster quarter
            NVC = BSH + KQ
            vcl_T = mapper_visual(b32p[:VD, 0:NVC], NVC, "vc")
            for mt in range(DT):
                ps = pspool.tile([128, 512], f32, tag="ps_map", name=f"vaps{mt}")
                for kt in range(DT):
                    nc.tensor.matmul(ps[:, :BSH], wslice(wa_sb, kt, mt),
                                     vcl_T[kt][:, :BSH],
                                     start=(kt == 0), stop=(kt == DT - 1))
                va16 = wpool.tile([128, BSH], f16, tag=f"va16_{mt}",
                                  name=f"va16_{mt}")
                nc.scalar.activation(va16[:], ps[:, :BSH], AF.Identity,
                                     bias=br1_sb[:, mt:mt + 1])
                nc.sync.dma_start(out=vach[mt * 128:(mt + 1) * 128, :], in_=va16[:])

            # semantic prototypes -> sem_pre_T
            rs1 = wpool.tile([128, DT * C], f32, tag="rs1")
            for mt in range(DT):
                ps = pspool.tile([128, 512], f32, tag="ps_map", name=f"sps{mt}")
                nc.tensor.matmul(ps[:, :C], ws1a_sb[:, mt * 128:(mt + 1) * 128],
                                 cpT0_sb[:], start=True, stop=False)
                nc.tensor.matmul(ps[:, :C], ws1b_sb[:SD - 128, mt * 128:(mt + 1) * 128],
                                 cpT1_sb[:SD - 128, :], start=False, stop=True)
                nc.scalar.activation(rs1[:, mt * C:(mt + 1) * C], ps[:, :C],
                                     AF.Relu, bias=bs1_sb[:, mt:mt + 1])
            # semp packed with the mapped clusters: [semp | cluc] [128, C+KQ]
            CK = C + KQ
            semp = []
            for mt in range(DT):
                ps = pspool.tile([128, 512], f32, tag="ps_map", name=f"sps2{mt}")
                for kt in range(DT):
                    nc.tensor.matmul(ps[:, :C], wslice(ws2_sb, kt, mt),
                                     rs1[:, kt * C:(kt + 1) * C],
                                     start=(kt == 0), stop=(kt == DT - 1))
                s = wpool.tile([128, CK], f32, tag=f"semp{mt}", name=f"semp{mt}")
                nc.scalar.activation(s[:, :C], ps[:, :C], AF.Identity,
                                     bias=bs2_sb[:, mt:mt + 1])
                nc.vector.tensor_copy(s[:, C:CK], vcl_T[mt][:, BSH:BSH + KQ])
                semp.append(s)

            # A''[e] (fp16) and Dm[e] from one matmul per (e, mt, kt)
            A16, Dm = [], []
            for e in range(EH):
                row_a, row_d = [], []
                for mt in range(DT):
                    ps = pspool.tile([128, 512], f32, tag="ps_map",
                                     name=f"aps{e}{mt}")
                    for kt in range(DT):
                        nc.tensor.matmul(ps[:, :CK], wslice(wexp_sb[e], kt, mt),
                                         semp[kt][:],
                                         start=(kt == 0), stop=(kt == DT - 1))
                    a = cpool.tile([128, C], f16, tag=f"A16_{e}_{mt}",
                                   name=f"A16_{e}_{mt}")
                    nc.scalar.activation(a[:], ps[:, :C], AF.Identity,
                                         bias=bexp_sb[:, e * DT + mt:e * DT + mt + 1])
                    d_t = cpool.tile([128, KQ], f32, tag=f"Dm{e}_{mt}",
                                     name=f"Dm{e}_{mt}")
                    nc.scalar.activation(d_t[:], ps[:, C:CK], AF.Identity,
                                         bias=0.0, scale=-1.0)
                    row_a.append(a)
                    row_d.append(d_t)
                A16.append(row_a)
                Dm.append(row_d)

            # block1 fused units + paired identity-matmul accumulation
            # 150 fused units; ~29% on ACT (Bresenham pattern), the rest
            # DVE.  Units are packed two-per-[128,512] tile; for the first
            # MERGE_QUADS quads per d'-tile the two tiles are pre-summed on
            # DVE (fp16 add) so one identity matmul accumulates 4 units --
            # this moves work from the PE (A's bottleneck) to DVE slack.
            MERGE_QUADS = 0
            N_ACT1 = 37
            units = [(e, k) for e in range(EH) for k in range(KQ)]
            pairs = [units[i:i + 2] for i in range(0, len(units), 2)]
            with tc.tile_pool(name="acc", bufs=1, space="PSUM") as accpool:
                pacc = [accpool.tile([128, 2 * C], f32, tag=f"pacc{t}",
                                     name=f"pacc{t}") for t in range(DT)]
                ucount = 0

                def emit_pair(t, pi, pair):
                    nonlocal ucount
                    hp = h1pool.tile([128, 2 * C], f16, tag="h1",
                                     name=f"h1_{t}_{pi}")
                    for s, (e, k) in enumerate(pair):
                        dst = hp[:, s * C:(s + 1) * C]
                        if (ucount * N_ACT1) % 150 < N_ACT1:
                            nc.scalar.activation(dst, A16[e][t][:], AF.Relu,
                                                 bias=Dm[e][t][:, k:k + 1])
                        else:
                            nc.vector.tensor_scalar(
                                dst, A16[e][t][:], Dm[e][t][:, k:k + 1], 0.0,
                                OP.add, OP.max)
                        ucount += 1
                    return hp, len(pair) * C

                for t in range(DT):
                    npairs = len(pairs)
                    first = True
                    pi = 0
                    while pi < npairs:
                        merge = (pi + 1 < npairs and pi // 2 < MERGE_QUADS
                                 and len(pairs[pi]) == 2 and len(pairs[pi + 1]) == 2)
                        hp, n = emit_pair(t, pi, pairs[pi])
                        if merge:
                            hp2, _ = emit_pair(t, pi + 1, pairs[pi + 1])
                            nc.vector.tensor_tensor(hp[:], hp[:], hp2[:], OP.add)
                            pi += 2
                        else:
                            pi += 1
                        nc.tensor.matmul(pacc[t][:, :n], idh_sb[:, :128],
                                         hp[:, :n], start=first,
                                         stop=(pi >= npairs),
                                         skip_group_check=True)
                        first = False
                for t in range(DT):
                    half = wpool.tile([128, C], f32, tag="half", name=f"half{t}")
                    nc.scalar.activation(half[:], pacc[t][:, C:2 * C], AF.Copy)
                    o = wpool.tile([128, C], f32, tag=f"o1_{t}", name=f"o1_{t}")
                    nc.vector.tensor_tensor(o[:], pacc[t][:, :C], half[:], OP.add)
                    nc.sync.dma_start(out=part[t * 128:(t + 1) * 128, :], in_=o[:])

    nc.compile()
    return nc


def _build_b():
    import concourse.bacc as bacc
    import concourse.mybir as mybir
    from concourse import tile

    f32, f16 = mybir.dt.float32, mybir.dt.float16
    AF, OP = mybir.ActivationFunctionType, mybir.AluOpType

    nc = bacc.Bacc("TRN2", target_bir_lowering=False, debug=False,
                   enable_asserts=False, num_devices=NCORES)
    vaTB = nc.dram_tensor("vaTB", [D, B], f16, kind="ExternalInput").ap()
    psl = nc.dram_tensor("psl", [NCORES, D, CSH], f32, kind="ExternalInput").ap()
    blob32b = nc.dram_tensor("blob32b", [128, F32B_W], f32,
                             kind="ExternalInput").ap()
    blob16b = nc.dram_tensor("blob16b", [128, F16B_W], f16,
                             kind="ExternalInput").ap()
    out2 = nc.dram_tensor("out2", [CSH, B], f32, kind="ExternalOutput").ap()

    with tile.TileContext(nc) as tc:
        with (
            tc.tile_pool(name="const", bufs=1) as cpool,
            tc.tile_pool(name="work", bufs=3) as wpool,
            tc.tile_pool(name="h2", bufs=12) as h2pool,
            tc.tile_pool(name="ps", bufs=2, space="PSUM") as pspool,
        ):
            b32b = cpool.tile([128, F32B_W], f32, tag="b32b")
            b16b = cpool.tile([128, F16B_W], f16, tag="b16b")
            _chunked_load(nc, b32b, blob32b, F32B_W)
            nc.sync.dma_start(out=b16b[:], in_=blob16b)
            Bc = lambda n: b32b[:, _F32B[n][0]:_F32B[n][0] + _F32B[n][1]]
            wb_sb, br2_sb = Bc("wb"), Bc("br2")
            w2h_sb = b16b[:, 0:DT * 32]

            vaT = []
            for t in range(DT):
                v = cpool.tile([128, B], f16, tag=f"vaT{t}", name=f"vaT{t}")
                nc.sync.dma_start(out=v[:], in_=vaTB[t * 128:(t + 1) * 128, :])
                vaT.append(v)

            # sum the 8 partial slices -> out1_mine_T [d' tiles][128, 32]
            pall = []
            for t in range(DT):
                pt = cpool.tile([128, NCORES * CSH], f32, tag=f"pall{t}",
                                name=f"pall{t}")
                nc.sync.dma_start(
                    out=pt[:].rearrange("p (c w) -> p c w", c=NCORES),
                    in_=psl.rearrange("c (u p) w -> u p c w", p=128)[t])
                pall.append(pt)
            omT = []
            for t in range(DT):
                s01 = wpool.tile([128, CSH], f32, tag="s01", name=f"s01_{t}")
                s23 = wpool.tile([128, CSH], f32, tag="s23", name=f"s23_{t}")
                s45 = wpool.tile([128, CSH], f32, tag="s45", name=f"s45_{t}")
                s67 = wpool.tile([128, CSH], f32, tag="s67", name=f"s67_{t}")
                W = CSH
                nc.vector.tensor_tensor(s01[:], pall[t][:, 0:W], pall[t][:, W:2 * W], OP.add)
                nc.vector.tensor_tensor(s23[:], pall[t][:, 2 * W:3 * W], pall[t][:, 3 * W:4 * W], OP.add)
                nc.scalar.activation(s45[:], pall[t][:, 4 * W:5 * W], AF.Identity,
                                     bias=0.0)
                nc.vector.tensor_tensor(s45[:], s45[:], pall[t][:, 5 * W:6 * W], OP.add)
                nc.vector.tensor_tensor(s67[:], pall[t][:, 6 * W:7 * W], pall[t][:, 7 * W:8 * W], OP.add)
                nc.vector.tensor_tensor(s01[:], s01[:], s23[:], OP.add)
                nc.vector.tensor_tensor(s45[:], s45[:], s67[:], OP.add)
                oT = wpool.tile([128, CSH], f32, tag=f"omT{t}", name=f"omT{t}")
                nc.vector.tensor_tensor(oT[:], s01[:], s45[:], OP.add)
                omT.append(oT)

            def wslice(wsb, kt, mt):
                return wsb[:, kt * D + mt * 128: kt * D + mt * 128 + 128]

            S2 = []
            for mt in range(DT):
                ps = pspool.tile([128, 512], f32, tag="ps_map", name=f"s2ps{mt}")
                for kt in range(DT):
                    nc.tensor.matmul(ps[:, :CSH], wslice(wb_sb, kt, mt),
                                     omT[kt][:], start=(kt == 0), stop=(kt == DT - 1))
                s2 = wpool.tile([128, CSH], f32, tag=f"S2_{mt}", name=f"S2_{mt}")
                nc.vector.tensor_copy(s2[:], ps[:, :CSH])
                S2.append(s2)

            with tc.tile_pool(name="psb2", bufs=3, space="PSUM") as psb2:
                ucount = 0
                for g in range(CSH // 4):
                    pg = psb2.tile([128, B], f32, tag="pg", name=f"pg{g}")
                    hh = {}
                    for j in range(4):
                        c = 4 * g + j
                        for t in range(DT):
                            h = h2pool.tile([128, B], f16, tag="h2",
                                            name=f"h2_{c}_{t}")
                            if (ucount * 14) % 64 < 14:
                                nc.scalar.activation(h[:], vaT[t][:], AF.Relu,
                                                     bias=S2[t][:, c:c + 1])
                            else:
                                nc.vector.tensor_scalar(
                                    h[:], vaT[t][:], S2[t][:, c:c + 1], 0.0,
                                    OP.add, OP.max)
                            hh[(j, t)] = h
                        ucount += 1
                    for ch in range(2):
                        for t in range(DT):
                            for j in range(4):
                                nc.tensor.matmul(
                                    pg[32 * j:32 * j + 32, ch * 512:(ch + 1) * 512],
                                    w2h_sb[:, t * 32:(t + 1) * 32],
                                    hh[(j, t)][:, ch * 512:(ch + 1) * 512],
                                    start=(t == 0), stop=(t == DT - 1),
                                    tile_position=(0, 32 * j),
                                    skip_group_check=True)
                    osb = cpool.tile([128, B], f32, tag=f"osb{g}", name=f"osb{g}")
                    if g in (0, 4):
                        nc.vector.tensor_scalar_add(osb[:], pg[:], br2_sb[:])
                    else:
                        nc.scalar.activation(osb[:], pg[:], AF.Identity,
                                             bias=br2_sb[:])
                    nc.sync.dma_start(
                        out=out2[4 * g:4 * g + 4, :],
                        in_=osb.rearrange("(s r) n -> s r n", r=32)[:, 0, :])

    nc.compile()
    return nc


def _prepare_a(inputs):
    f = lambda x: np.ascontiguousarray(x, dtype=np.float32)
    vf, cc = f(inputs["visual_features"]), f(inputs["cluster_centers"])
    cpT = f(inputs["class_prototypes"]).T
    W_r1, W_exp, b_exp = f(inputs["W_r1"]), f(inputs["W_exp"]), f(inputs["b_exp"])

    def pad128(x):
        out = np.zeros((128, x.shape[1]), np.float32)
        out[:x.shape[0]] = x
        return out

    w2t = lambda w: np.concatenate([w[:128], w[128:]], axis=1)
    b2 = lambda b: np.ascontiguousarray(f(b).reshape(DT, 128).T)

    blob = np.zeros((128, F32A_W), np.float32)

    def put(name, arr):
        o, w = _F32A[name]
        blob[:, o:o + w] = arr

    put("wv1", pad128(f(inputs["W_v1"])))
    put("wv2", w2t(f(inputs["W_v2"])))
    put("ws2", w2t(f(inputs["W_s2"])))
    put("wa", w2t(W_r1[:D]))
    ws1 = f(inputs["W_s1"])
    put("ws1a", ws1[:128])
    put("ws1b", pad128(ws1[128:]))
    put("cpT0", cpT[:128])
    put("cpT1", pad128(cpT[128:]))
    for nm, key in [("bv1", "b_v1"), ("bv2", "b_v2"), ("bs1", "b_s1"),
                    ("bs2", "b_s2"), ("br1", "b_r1")]:
        put(nm, b2(inputs[key]))
    idh = np.eye(128, dtype=np.float16)

    in_maps = []
    for i in range(NCORES):
        h, q = i // 4, i % 4
        bp = np.zeros((128, F32P_W), np.float32)

        def putp(name, arr):
            o, w = _F32P[name]
            bp[:, o:o + w] = arr

        putp("vfT", pad128(np.ascontiguousarray(vf[BSH * i:BSH * (i + 1)].T)))
        putp("cluT", pad128(np.ascontiguousarray(cc[KQ * q:KQ * (q + 1)].T)))
        for e in range(EH):
            putp(f"wexp{e}", w2t(W_exp[EH * h + e]))
        putp("bexp", np.ascontiguousarray(
            b_exp[EH * h:EH * h + EH].reshape(EH * DT, 128).T))
        in_maps.append(dict(blob32a=blob, blob32p=bp, idh16=idh))
    return in_maps


def _prepare_b(inputs, res_a):
    f = lambda x: np.ascontiguousarray(x, dtype=np.float32)
    W_r1 = f(inputs["W_r1"])
    blob = np.zeros((128, F32B_W), np.float32)
    o, w = _F32B["wb"]
    blob[:, o:o + w] = np.concatenate([W_r1[D:D + 128], W_r1[D + 128:]], axis=1)
    o, w = _F32B["br2"]
    blob[:, o:o + w] = np.full((128, 1), float(np.asarray(inputs["b_r2"]).reshape(-1)[0]),
                               np.float32)
    blob16 = np.zeros((128, F16B_W), np.float16)
    o, w = _F16B["w2h"]
    blob16[:, o:o + w] = np.repeat(
        f(inputs["W_r2"]).reshape(DT, 128).T.astype(np.float16), 32, axis=1)

    vaTB = np.concatenate([res_a[i]["vach"] for i in range(NCORES)], axis=1)
    parts = np.stack([res_a[i]["part"] for i in range(NCORES)])  # [8, D, C]
    in_maps = []
    for i in range(NCORES):
        in_maps.append(dict(
            vaTB=vaTB,
            psl=np.ascontiguousarray(parts[:, :, CSH * i:CSH * (i + 1)]),
            blob32b=blob, blob16b=blob16))
    return in_maps


def _assemble(results):
    cols = np.concatenate([results[i]["out2"] for i in range(NCORES)], axis=0)
    return np.ascontiguousarray(cols.T, dtype=np.float32)  # [B, C]


_CACHED = {}


def run_two_phase(inputs, trace=False, **kw):
    from concourse.bass_utils import run_bass_kernel_spmd
    if "nca" not in _CACHED:
        _CACHED["nca"] = _build_a()
        _CACHED["ncb"] = _build_b()
    cores = list(range(NCORES))
    ra = run_bass_kernel_spmd(_CACHED["nca"], _prepare_a(inputs), cores,
                              trace=trace, **kw)
    rb = run_bass_kernel_spmd(_CACHED["ncb"], _prepare_b(inputs, ra.results),
                              cores, trace=trace, **kw)
    return _assemble(rb.results), ra, rb


def kernel(**inputs) -> np.ndarray:
    out, _, _ = run_two_phase(inputs, trace=False)
    return out

